# revision 1
# baseline (speedup 1.0000x reference)
"""Modulated deformable conv (DCNv2) on 8 Trainium2 NeuronCores.

Sharding: 8 shards = (batch b in 0..4) x (row-half rh in 0..2); each core
computes 40 output rows x 256 channels of one batch image.

Per-core pipeline (all compute on device):
  P1 offset conv (3x3, 256->27) as PE matmuls, fp32
  P2 PE-transpose offsets to pixel-major
  P3 DVE field chain: bilinear corner weights (sigmoid mask folded in) +
     int16 gather indices into a zero-padded pixel-major fp16 image
     (the zero padding implements out-of-bounds corner masking exactly)
  P4 dma_gather: per (tap,pixel) fetch the two 2-pixel corner row-pairs
  P5 DVE modulate: 4 fused per-partition-scalar MACs per 128-sample slot
  P6 PE-transpose modulated samples pixel-major -> channel-major rhs
  P7 PE matmuls contract (c, tap) into PSUM (36-matmul accumulation)
  P8 stage + DMA out
"""
import os
import sys

if "/opt/trn_rl_repo" not in sys.path:
    sys.path.insert(0, "/opt/trn_rl_repo")

# The device path needs the axon PJRT platform; a harness that pins
# JAX_PLATFORMS=cpu would hide the NeuronCores.
if os.environ.get("JAX_PLATFORMS") and "axon" not in os.environ["JAX_PLATFORMS"]:
    os.environ["JAX_PLATFORMS"] = "axon," + os.environ["JAX_PLATFORMS"]

from contextlib import ExitStack

import numpy as np

import concourse.bacc as bacc
import concourse.bass as bass
import concourse.mybir as mybir
import concourse.tile as tile
from concourse import bass_utils

F32 = mybir.dt.float32
F16 = mybir.dt.float16
I16 = mybir.dt.int16
AOP = mybir.AluOpType
ACTF = mybir.ActivationFunctionType

B, C, H, W = 4, 256, 80, 80
K = 9
PAD, PW = 5, 90
NPIX = PW * PW
ROWS = 40
QO = 3584              # per-tap pixel stream: 40*82=3280 padded to 28*128
SLOTS = 28
S_ALL = K * SLOTS      # 252
L = K * QO             # 32256
CH_SLOTS = 7           # quarter-tap chunks
CHN = CH_SLOTS * 128   # 896
NBW = 448              # matmul N (2 blocks per chunk)
XF = 3776

_CACHE = {}
KVARIANT = os.environ.get("KVARIANT", "")
SP1 = "nosp1" not in KVARIANT
LOOPN = int(os.environ.get("LOOPN", "1"))


def _emit(tc, outs, ins):
    nc = tc.nc
    ctx = ExitStack()
    out_dram = outs["out"]

    cpool = ctx.enter_context(tc.tile_pool(name="consts", bufs=1))
    wpool = ctx.enter_context(tc.tile_pool(name="work", bufs=1))

    xcm = cpool.tile([128, 2, XF], F32)
    dwt = cpool.tile([128, 36, 128], F16)
    offw = cpool.tile([128, 2, K, 27], F32)
    offb = cpool.tile([27, 1], F32)
    ident = cpool.tile([27, 27], F32)
    ident128 = cpool.tile([128, 128], F16)
    ybase = cpool.tile([128, S_ALL], F32)
    xbase = cpool.tile([128, S_ALL], F32)
    zb = cpool.tile([128, 1], F32)

    nc.sync.dma_start(xcm[:], ins["xcm"][:])
    nc.sync.dma_start(dwt[:], ins["dwt"][:])
    nc.sync.dma_start(offw[:], ins["offw"][:])
    nc.sync.dma_start(offb[:], ins["offb"][:])
    nc.sync.dma_start(ident[:], ins["ident27"][:])
    nc.sync.dma_start(ident128[:], ins["ident128"][:])
    nc.sync.dma_start(ybase[:], ins["ybase"][:])
    nc.sync.dma_start(xbase[:], ins["xbase"][:])
    nc.gpsimd.memset(zb[:], 0.0)

    offs = wpool.tile([27, QO], F32)
    offsT = wpool.tile([128, SLOTS * 27], F32)
    NF = 10
    FB = wpool.tile([128, NF, S_ALL], F32)
    (pys, fy, gy, pxs, fx, gx, msig, tA, tB, tC) = tuple(FB[:, i, :] for i in range(NF))
    CW = wpool.tile([128, 4, S_ALL], F32)
    IDXF = wpool.tile([128, 2, S_ALL], I16)
    ST = wpool.tile([128, 2, L // 16], I16)
    staging = wpool.tile([128, 2, QO], F32)

    # ---- P1: offset conv ----
    for _rep in range(LOOPN):
      with tc.tile_pool(name="psum1", bufs=2, space="PSUM") as pp1:
          for blk in range(7):
              q0 = blk * 512
              po = pp1.tile([27, 512], F32, tag="po", name=f"po_{_rep}_{blk}")
              first = True
              for k in range(K):
                  for ch in range(2):
                      f0 = 82 * (k // 3) + (k % 3)
                      nc.tensor.matmul(
                          po[:], offw[:, ch, k, :],
                          xcm[:, ch, f0 + q0:f0 + q0 + 512],
                          start=first, stop=(ch == 1 and k == K - 1),
                      )
                      first = False
              nc.vector.tensor_scalar(
                  offs[:, q0:q0 + 512], po[:], offb[:, 0:1], None, AOP.add
              )

          # ---- P2: transpose offsets to pixel-major (14 blocks per bank) ----
          for half in range(2):
              po2 = pp1.tile([128, 14 * 27], F32, tag="poT", name=f"po2_{_rep}_{half}")
              for jj in range(14):
                  j = half * 14 + jj
                  nc.tensor.transpose(
                      po2[:, jj * 27:(jj + 1) * 27], offs[:, j * 128:(j + 1) * 128],
                      ident[:],
                  )
              nc.vector.tensor_copy(offsT[:, half * 378:(half + 1) * 378], po2[:])

      # ---- P3: fields ----
      r = offsT[:].rearrange("p (s c) -> p c s", c=27)
      dy, dx, ml = r[:, 0:K, :], r[:, K:2 * K, :], r[:, 2 * K:3 * K, :]

      def v3(ap):
          return ap.rearrange("p (k s) -> p k s", k=K)

      V = nc.vector
      BIGF = 8388608.0  # 2^23: (x + BIGF) - BIGF == round-to-nearest-int(x) in fp32
      V.tensor_add(v3(pys), v3(ybase[:]), dy)
      V.tensor_scalar(v3(tC), v3(pys), BIGF, None, AOP.add)
      V.tensor_scalar(v3(tC), v3(tC), -BIGF, None, AOP.add)
      V.tensor_tensor(v3(tB), v3(tC), v3(pys), AOP.is_gt)
      V.tensor_sub(v3(tA), v3(tC), v3(tB))               # tA = y0s = floor(pys)
      V.tensor_sub(v3(fy), v3(pys), v3(tA))
      V.tensor_scalar(v3(gy), v3(fy), -1.0, 1.0, AOP.mult, op1=AOP.add)
      V.tensor_add(v3(pxs), v3(xbase[:]), dx)
      V.tensor_scalar(v3(tC), v3(pxs), BIGF, None, AOP.add)
      V.tensor_scalar(v3(tC), v3(tC), -BIGF, None, AOP.add)
      V.tensor_tensor(v3(msig), v3(tC), v3(pxs), AOP.is_gt)
      V.tensor_sub(v3(tB), v3(tC), v3(msig))             # tB = x0s = floor(pxs)
      V.tensor_sub(v3(fx), v3(pxs), v3(tB))
      V.tensor_scalar(v3(gx), v3(fx), -1.0, 1.0, AOP.mult, op1=AOP.add)
      nc.scalar.activation(v3(msig), ml, ACTF.Sigmoid, bias=zb[:])
      V.tensor_mul(v3(pys), v3(gy), v3(msig))            # wy0*m
      V.tensor_mul(v3(pxs), v3(fy), v3(msig))            # wy1*m
      V.tensor_mul(v3(CW[:, 0, :]), v3(pys), v3(gx))
      V.tensor_mul(v3(CW[:, 1, :]), v3(pys), v3(fx))
      V.tensor_mul(v3(CW[:, 2, :]), v3(pxs), v3(gx))
      V.tensor_mul(v3(CW[:, 3, :]), v3(pxs), v3(fx))
      V.tensor_scalar(v3(tA), v3(tA), 59.0, 147.0, AOP.max, op1=AOP.min)
      V.tensor_scalar(v3(tB), v3(tB), 59.0, 147.0, AOP.max, op1=AOP.min)
      V.tensor_scalar(v3(tB), v3(tB), -5369.0, None, AOP.add)
      V.scalar_tensor_tensor(v3(tC), v3(tA), 90.0, v3(tB), AOP.mult, AOP.add)
      V.tensor_copy(v3(IDXF[:, 0, :]), v3(tC))
      V.tensor_scalar(v3(tC), v3(tC), 90.0, None, AOP.add)
      V.tensor_copy(v3(IDXF[:, 1, :]), v3(tC))

      # streams: pos i = s*128+p lives at (i%16, i//16); replicate to 8 Q7 blocks
      for t in range(2):
          dst = ST[0:16, t, :].rearrange("p (s e) -> p e s", e=8)
          for bq in range(8):
              nc.sync.dma_start(dst[:, bq, :], IDXF[bq * 16:(bq + 1) * 16, t, :])
      for rep in range(1, 8):
          nc.sync.dma_start(ST[rep * 16:(rep + 1) * 16, :, :], ST[0:16, :, :])

      # ---- main loop: quarter-tap chunks ----
      xg_ap = bass.AP(ins["xpm"].tensor, 0, [[256, 8100], [1, 512]])

      mctx = ExitStack()
      gpool = mctx.enter_context(tc.tile_pool(name=f"g{_rep}", bufs=3))
      upool = mctx.enter_context(tc.tile_pool(name=f"u{_rep}", bufs=3))
      tpool = mctx.enter_context(tc.tile_pool(name=f"tmp{_rep}", bufs=8))
      rpool = mctx.enter_context(tc.tile_pool(name=f"rhs{_rep}", bufs=3))
      pp2 = mctx.enter_context(tc.tile_pool(name=f"psum2{_rep}", bufs=1, space="PSUM"))
      ppT = mctx.enter_context(tc.tile_pool(name=f"psumT{_rep}", bufs=2, space="PSUM"))

      for h4 in range(4):
          ps = [[pp2.tile([128, NBW], F32, tag=f"ps{coh}{nb}", name=f"ps{coh}{nb}_{_rep}_{h4}")
                 for nb in range(2)] for coh in range(2)]
          for k in range(K):
              s0 = k * SLOTS + h4 * CH_SLOTS
              j0 = s0 * 8
              g = gpool.tile([128, 2, CH_SLOTS, 512], F16, tag="g", name=f"g_{_rep}_{h4}_{k}")
              if "nog1" in KVARIANT:
                  nc.gpsimd.memset(g[:], 0.25)
              else:
                  for t in range(2):
                      nc.gpsimd.dma_gather(
                          g[:, t, :, :], xg_ap, ST[:, t, j0:j0 + CH_SLOTS * 8],
                          CHN, CHN, 512, elem_step=256, single_packet=SP1,
                      )
              u = upool.tile([128, CH_SLOTS, 256], F16, tag="u", name=f"u_{_rep}_{h4}_{k}")
              if "nomod" in KVARIANT:
                  nc.vector.tensor_copy(u[:], g[:, 0, :, 0:256])
              else:
                  for sl in range(CH_SLOTS):
                      s = s0 + sl
                      t0 = tpool.tile([128, 256], F16, tag="t0", name=f"t0_{_rep}_{h4}_{k}_{sl}")
                      t1 = tpool.tile([128, 256], F16, tag="t1", name=f"t1_{_rep}_{h4}_{k}_{sl}")
                      if "noactmod" not in KVARIANT:
                          nc.scalar.activation(t0[:], g[:, 0, sl, 0:256], ACTF.Copy,
                                               bias=0.0, scale=CW[:, 0, s:s + 1])
                      else:
                          V.tensor_scalar(t0[:], g[:, 0, sl, 0:256], CW[:, 0, s:s + 1], None, AOP.mult)
                      V.scalar_tensor_tensor(t1[:], g[:, 0, sl, 256:512], CW[:, 1, s:s + 1], t0[:], AOP.mult, AOP.add)
                      V.scalar_tensor_tensor(t0[:], g[:, 1, sl, 0:256], CW[:, 2, s:s + 1], t1[:], AOP.mult, AOP.add)
                      V.scalar_tensor_tensor(u[:, sl, :], g[:, 1, sl, 256:512], CW[:, 3, s:s + 1], t0[:], AOP.mult, AOP.add)
              # P6: PE-transpose u -> channel-major rhs
              psT = ppT.tile([128, 14, 128], F16, tag="psT", name=f"psT_{_rep}_{h4}_{k}")
              rhs = rpool.tile([128, 2, CHN], F16, tag="rhs", name=f"rhs_{_rep}_{h4}_{k}")
              for sl in range(CH_SLOTS):
                  for m in range(2):
                      nc.tensor.transpose(
                          psT[:, sl * 2 + m, :], u[:, sl, m * 128:(m + 1) * 128],
                          ident128[:],
                      )
              psTv = psT[:].rearrange("p (sl m) c -> p m sl c", m=2)
              for m in range(2):
                  nc.scalar.activation(
                      rhs[:, m, :].rearrange("p (sl c) -> p sl c", c=128),
                      psTv[:, m], ACTF.Copy, bias=0.0,
                  )
              for mm in range(2):
                  for coh in range(2):
                      for nb in range(2):
                          nc.tensor.matmul(
                              ps[coh][nb][:],
                              dwt[:, (k * 2 + mm) * 2 + coh, :],
                              rhs[:, mm, nb * NBW:(nb + 1) * NBW],
                              start=(k == 0 and mm == 0),
                              stop=(k == K - 1 and mm == 1),
                          )
          for coh in range(2):
              for nb in range(2):
                  nc.scalar.activation(
                      staging[:, coh, h4 * CHN + nb * NBW:h4 * CHN + (nb + 1) * NBW],
                      ps[coh][nb][:], ACTF.Copy, bias=0.0,
                  )

      for coh in range(2):
          src = staging[:, coh, 0:3280].rearrange("p (y x) -> p y x", x=82)
          nc.sync.dma_start(out_dram[coh], src[:, :, 1:81])
      mctx.close()
    ctx.close()


def _build():
    key = ("nc", KVARIANT, LOOPN)
    if key in _CACHE:
        return _CACHE[key]
    nc = bacc.Bacc("TRN2", target_bir_lowering=False, debug=False, num_devices=8)
    ins = {
        "xcm": nc.dram_tensor("xcm", [128, 2, XF], F32, kind="ExternalInput").ap(),
        "xpm": nc.dram_tensor("xpm", [NPIX + 1, 256], F16, kind="ExternalInput").ap(),
        "offw": nc.dram_tensor("offw", [128, 2, K, 27], F32, kind="ExternalInput").ap(),
        "offb": nc.dram_tensor("offb", [27, 1], F32, kind="ExternalInput").ap(),
        "dwt": nc.dram_tensor("dwt", [128, 36, 128], F16, kind="ExternalInput").ap(),
        "ybase": nc.dram_tensor("ybase", [128, S_ALL], F32, kind="ExternalInput").ap(),
        "xbase": nc.dram_tensor("xbase", [128, S_ALL], F32, kind="ExternalInput").ap(),
        "ident27": nc.dram_tensor("ident27", [27, 27], F32, kind="ExternalInput").ap(),
        "ident128": nc.dram_tensor("ident128", [128, 128], F16, kind="ExternalInput").ap(),
    }
    outs = {"out": nc.dram_tensor("out", [2, 128, ROWS, W], F32, kind="ExternalOutput").ap()}
    with tile.TileContext(nc) as tc:
        _emit(tc, outs, ins)
    nc.compile()
    _CACHE[key] = nc
    return nc


def _host_prep(x, offset_w, offset_b, deform_w):
    prep = {}
    xp = np.zeros((B, PW, PW, C), np.float16)
    xp[:, PAD:PAD + H, PAD:PAD + W, :] = np.transpose(x, (0, 2, 3, 1)).astype(np.float16)
    xpm = np.zeros((B, NPIX + 1, C), np.float16)
    xpm[:, :NPIX] = xp.reshape(B, NPIX, C)
    prep["xpm"] = xpm

    xcm = np.zeros((B, 2, 128, 2, XF), np.float32)
    for rh in range(2):
        y0c = rh * ROWS
        for rl in range(46):
            ty = y0c - 1 + rl
            if ty < 0 or ty >= H:
                continue
            f0 = 1 + rl * 82
            if f0 + 1 >= XF:
                continue
            seg = min(80, XF - f0 - 1)
            if seg <= 0:
                continue
            v = x[:, :, ty, :seg]
            xcm[:, rh, :, 0, f0 + 1:f0 + 1 + seg] = v[:, :128]
            xcm[:, rh, :, 1, f0 + 1:f0 + 1 + seg] = v[:, 128:]
    prep["xcm"] = xcm

    offw = np.zeros((128, 2, K, 27), np.float32)
    for ch in range(2):
        for k in range(K):
            offw[:, ch, k, :] = offset_w[:, ch * 128:(ch + 1) * 128, k // 3, k % 3].T
    prep["offw"] = offw
    prep["offb"] = np.ascontiguousarray(offset_b.reshape(27, 1).astype(np.float32))

    dwt = np.zeros((128, 36, 128), np.float16)
    for k in range(K):
        for mm in range(2):
            for coh in range(2):
                dwt[:, (k * 2 + mm) * 2 + coh, :] = deform_w[
                    coh * 128:(coh + 1) * 128, mm * 128:(mm + 1) * 128, k // 3, k % 3
                ].T.astype(np.float16)
    prep["dwt"] = dwt

    ybase = np.zeros((2, 128, S_ALL), np.float32)
    xbase = np.zeros((2, 128, S_ALL), np.float32)
    p_i = np.arange(128)
    for rh in range(2):
        for s in range(S_ALL):
            k, sp = s // SLOTS, s % SLOTS
            q = sp * 128 + p_i
            ybase[rh, :, s] = rh * ROWS + q // 82 + (k // 3 - 1) + 64
            xbase[rh, :, s] = (q % 82) - 1 + (k % 3 - 1) + 64
    prep["ybase"] = ybase
    prep["xbase"] = xbase

    prep["ident27"] = np.eye(27, dtype=np.float32)
    prep["ident128"] = np.eye(128, dtype=np.float16)
    return prep


def _in_maps(prep):
    maps = []
    for core in range(8):
        b, rh = core // 2, core % 2
        maps.append({
            "xcm": np.ascontiguousarray(prep["xcm"][b, rh]),
            "xpm": np.ascontiguousarray(prep["xpm"][b]),
            "offw": prep["offw"], "offb": prep["offb"], "dwt": prep["dwt"],
            "ybase": np.ascontiguousarray(prep["ybase"][rh]),
            "xbase": np.ascontiguousarray(prep["xbase"][rh]),
            "ident27": prep["ident27"], "ident128": prep["ident128"],
        })
    return maps


def kernel(x, offset_w, offset_b, deform_w, _profile=False, **_kw):
    x = np.asarray(x, np.float32)
    offset_w = np.asarray(offset_w, np.float32)
    offset_b = np.asarray(offset_b, np.float32)
    deform_w = np.asarray(deform_w, np.float32)

    nc = _build()
    prep = _host_prep(x, offset_w, offset_b, deform_w)
    res = bass_utils.run_bass_kernel_spmd(nc, _in_maps(prep), list(range(8)), trace=_profile)
    out = np.zeros((B, C, H, W), np.float32)
    for core in range(8):
        b, rh = core // 2, core % 2
        o = res.results[core]["out"]
        out[b, :128, rh * ROWS:(rh + 1) * ROWS] = o[0]
        out[b, 128:, rh * ROWS:(rh + 1) * ROWS] = o[1]
    if _profile:
        return out, res
    return out

